# revision 10
# baseline (speedup 1.0000x reference)
"""Trainium2 Bass kernel for binary-weight multi-head attention (nn_BMHA).

Reference computation (B=2, S=2048, D=1024, H=16, hd=64):
  q,k,v = hidden @ sign(w{q,k,v}).T  -> split heads
  scores = q@k.T/8, mask keys, probs = softmax(scores)
  attn = probs@v ; output = attn @ sign(wo).T
  returns (output, probs)

Sharding: 8 cores; core c owns heads [2c, 2c+2) for both batches.
  wq/wk/wv column-sharded, wo row-sharded; probs shards on the head axis;
  partial outputs summed on host.
"""
import os
import sys

import numpy as np

sys.path.insert(0, "/opt/trn_rl_repo")

B, S, D, H = 2, 2048, 1024, 16
NCORES = 8
HPC = H // NCORES          # heads per core = 2
DPC = HPC * (D // H)       # dims per core = 128
NQT = S // 128             # query tiles = 16
NKC = S // 512             # key chunks per psum row = 4
NKT = S // 128             # key tiles = 16

_cache = {}


def _build(reps=1):
    import concourse.bass as bass
    import concourse.mybir as mybir
    import concourse.tile as tile
    from concourse import bacc

    F32 = mybir.dt.float32
    F32R = mybir.dt.float32r
    BF16 = mybir.dt.bfloat16
    EXP = mybir.ActivationFunctionType.Exp

    nc = bacc.Bacc("TRN2", target_bir_lowering=False, debug=False,
                   num_devices=NCORES)

    hT_d = nc.dram_tensor("hT", [B, D, S], F32, kind="ExternalInput").ap()
    wqT_d = nc.dram_tensor("wqT", [D, DPC], F32, kind="ExternalInput").ap()
    wkT_d = nc.dram_tensor("wkT", [D, DPC], F32, kind="ExternalInput").ap()
    wvT_d = nc.dram_tensor("wvT", [D, DPC], F32, kind="ExternalInput").ap()
    woT_d = nc.dram_tensor("woT", [DPC, D], F32R, kind="ExternalInput").ap()
    mk_d = nc.dram_tensor("mk", [B, 128, S], F32, kind="ExternalInput").ap()
    probs_d = nc.dram_tensor("probs", [B, HPC, S, S], F32,
                             kind="ExternalOutput").ap()
    out_d = nc.dram_tensor("out", [B, S, D], F32, kind="ExternalOutput").ap()

    from concourse.masks import make_identity

    with tile.TileContext(nc) as tc:
        with tc.tile_pool(name="per", bufs=1) as per, \
             tc.tile_pool(name="ht", bufs=4) as htp, \
             tc.tile_pool(name="qkv", bufs=1) as qkvp, \
             tc.tile_pool(name="pt", bufs=1) as ptp, \
             tc.tile_pool(name="pr", bufs=2) as prp, \
             tc.tile_pool(name="st", bufs=4) as stp, \
             tc.tile_pool(name="sps", bufs=1, space="PSUM") as sps, \
             tc.tile_pool(name="mps", bufs=2, space="PSUM") as mps:

            ident = per.tile([128, 128], BF16, tag="ident")
            make_identity(nc, ident)
            woT = per.tile([DPC, D], F32R, tag="woT")
            nc.sync.dma_start(woT, woT_d)
            wts = []
            for wi, wslice in enumerate((wqT_d, wkT_d, wvT_d)):
                wt = per.tile([128, 8 * DPC], F32, tag=f"wt{wi}",
                              name=f"wt{wi}")
                nc.sync.dma_start(
                    wt.rearrange("p (t m) -> p t m", t=8),
                    wslice.rearrange("(t p) m -> p t m", p=128))
                wts.append(wt)

            for b in [b for _ in range(reps) for b in range(B)]:
                # ---- phase 1: QKV projections for batch b ----
                qT = qkvp.tile([128, S], F32, tag="qT")
                kT = qkvp.tile([128, S], F32, tag="kT")
                vT = qkvp.tile([128, S], F32, tag="vT")
                mk = qkvp.tile([128, S], F32, tag="mk")
                nc.sync.dma_start(mk, mk_d[b])
                # q and k together, half the seq columns at a time
                for half in range(2):
                    hb = half * 1024
                    ps = sps.tile([128, S], F32, tag="big")
                    for kd in range(8):
                        ht = htp.tile([128, 1024], F32, tag="ht",
                                      padded_shape=[128, S])
                        nc.sync.dma_start(
                            ht, hT_d[b, kd * 128:(kd + 1) * 128,
                                     hb:hb + 1024])
                        for pi, wt in enumerate(wts[:2]):
                            for c in range(2):
                                nc.tensor.matmul(
                                    ps[:, pi * 1024 + c * 512:
                                       pi * 1024 + (c + 1) * 512],
                                    wt[:, kd * DPC:(kd + 1) * DPC],
                                    ht[:, c * 512:(c + 1) * 512],
                                    start=(kd == 0), stop=(kd == 7))
                    for pi, dst in enumerate((qT, kT)):
                        for c in range(2):
                            nc.scalar.copy(
                                dst[:, hb + c * 512:hb + (c + 1) * 512],
                                ps[:, pi * 1024 + c * 512:
                                   pi * 1024 + (c + 1) * 512])
                # v in one full sweep
                ps = sps.tile([128, S], F32, tag="big")
                for kd in range(8):
                    ht = htp.tile([128, S], F32, tag="ht")
                    nc.sync.dma_start(
                        ht, hT_d[b, kd * 128:(kd + 1) * 128, :])
                    for c in range(NKC):
                        nc.tensor.matmul(
                            ps[:, c * 512:(c + 1) * 512],
                            wts[2][:, kd * DPC:(kd + 1) * DPC],
                            ht[:, c * 512:(c + 1) * 512],
                            start=(kd == 0), stop=(kd == 7))
                for c in range(NKC):
                    nc.scalar.copy(vT[:, c * 512:(c + 1) * 512],
                                   ps[:, c * 512:(c + 1) * 512])
                # mask the key projections (zero masked columns)
                nc.vector.tensor_tensor(kT, kT, mk, op=mybir.AluOpType.mult)
                # bf16 v in natural (key-major) layout for probs@v
                vbf = qkvp.tile([128, S], BF16, tag="vbf")
                nc.vector.tensor_copy(vbf, vT)
                vns = []
                for j in range(NKT):
                    tp = mps.tile([128, 128], BF16, tag="mps")
                    nc.tensor.transpose(tp, vbf[:, j * 128:(j + 1) * 128], ident)
                    vn = qkvp.tile([128, 128], BF16, tag=f"vn{j}")
                    nc.vector.tensor_copy(vn, tp)
                    vns.append(vn)

                attnT = qkvp.tile([128, S], F32R, tag="attnT")

                for hl in range(HPC):
                    hs = hl * 64
                    # probsT tiles for this (b, head)
                    pts = [ptp.tile([128, S], BF16, tag=f"pt{j}",
                                    name=f"pt{j}")
                           for j in range(NKT)]
                    for qi in range(NQT):
                        qs = qi * 128
                        sc = sps.tile([128, S], F32, tag="big")
                        for c in range(NKC):
                            nc.tensor.matmul(
                                sc[:, c * 512:(c + 1) * 512],
                                qT[hs:hs + 64, qs:qs + 128],
                                kT[hs:hs + 64, c * 512:(c + 1) * 512],
                                start=True, stop=True)
                        nmax = stp.tile([128, 1], F32, tag="nmax")
                        nc.vector.tensor_reduce(
                            nmax, sc, axis=mybir.AxisListType.X,
                            op=mybir.AluOpType.max, negate=True)
                        nmax8 = stp.tile([128, 1], F32, tag="nmax8")
                        nc.scalar.mul(nmax8, nmax, 0.125)
                        pe = prp.tile([128, S], F32, tag="pe")
                        z = stp.tile([128, 1], F32, tag="z")
                        nc.scalar.activation(pe, sc, EXP, bias=nmax8,
                                             scale=0.125, accum_out=z)
                        rz = stp.tile([128, 1], F32, tag="rz")
                        nc.vector.reciprocal(rz, z)
                        pb = prp.tile([128, S], BF16, tag="pb")
                        nc.vector.tensor_scalar_mul(pb, pe, rz)
                        nc.vector.tensor_scalar_mul(pe, pe, rz)
                        nc.sync.dma_start(
                            probs_d[b, hl, qs:qs + 128, :], pe)
                        for j in range(NKT):
                            nc.sync.dma_start_transpose(
                                pts[j][:, qs:qs + 128],
                                pb[:, j * 128:(j + 1) * 128])
                    # attnT[e, i] accumulation over key tiles
                    for ic in range(NKC):
                        aps = mps.tile([64, 512], F32, tag="mps")
                        for j in range(NKT):
                            nc.tensor.matmul(
                                aps, vns[j][:, hs:hs + 64],
                                pts[j][:, ic * 512:(ic + 1) * 512],
                                start=(j == 0), stop=(j == NKT - 1))
                        nc.vector.tensor_copy(
                            attnT[hs:hs + 64, ic * 512:(ic + 1) * 512], aps)

                # ---- phase 3: output projection (fp32r) ----
                for si in range(NQT):
                    ss = si * 128
                    for c in range(2):
                        ops = mps.tile([128, 512], F32, tag="mps")
                        nc.tensor.matmul(
                            ops, attnT[:, ss:ss + 128],
                            woT[:, c * 512:(c + 1) * 512],
                            start=True, stop=True)
                        ob = stp.tile([128, 512], F32, tag="ob")
                        nc.vector.tensor_copy(ob, ops)
                        nc.sync.dma_start(
                            out_d[b, ss:ss + 128, c * 512:(c + 1) * 512], ob)
    return nc


def kernel(hidden_states, mask, wq, wk, wv, wo):
    from concourse import bass_utils

    hidden_states = np.asarray(hidden_states, dtype=np.float32)
    mask = np.asarray(mask)
    sgn = lambda w: np.where(np.asarray(w) >= 0, np.float32(1.0),
                             np.float32(-1.0))
    wqb, wkb, wvb, wob = sgn(wq), sgn(wk), sgn(wv), sgn(wo)

    hT = np.ascontiguousarray(hidden_states.transpose(0, 2, 1))
    mkf = np.ascontiguousarray(
        np.broadcast_to(mask.astype(np.float32)[:, None, :], (B, 128, S)))

    in_maps = []
    for c in range(NCORES):
        sl = slice(c * DPC, (c + 1) * DPC)
        in_maps.append({
            "hT": hT,
            "wqT": np.ascontiguousarray(wqb[sl, :].T),
            "wkT": np.ascontiguousarray(wkb[sl, :].T),
            "wvT": np.ascontiguousarray(wvb[sl, :].T),
            "woT": np.ascontiguousarray(wob[:, sl].T),
            "mk": mkf,
        })

    if "nc" not in _cache:
        nc = _build()
        nc.compile()
        _cache["nc"] = nc
    nc = _cache["nc"]

    res = bass_utils.run_bass_kernel_spmd(
        nc, in_maps, core_ids=list(range(NCORES)),
        **_cache.get("run_kwargs", {}))
    _cache["last_result"] = res

    probs = np.empty((B, H, S, S), dtype=np.float32)
    output = np.zeros((B, S, D), dtype=np.float32)
    for c in range(NCORES):
        r = res.results[c]
        probs[:, c * HPC:(c + 1) * HPC] = r["probs"]
        output += r["out"]
    return output, probs
